# revision 1
# baseline (speedup 1.0000x reference)
"""Trainium2 Bass kernel for a directed MPNN layer (8 NeuronCores, SPMD).

Reference computation (per edge e = (src, tgt)):
    msg  = relu(edge_hidden @ W_msg.T + b_msg)                     (E, H)
    agg  = segment_sum(msg, tgt, N)                                (N, H)
    excl[e] = sum msg[f] over f with (tgt_f, src_f) == (src_e, tgt_e)
    out[e]  = relu(x[src_e] @ Wx.T + edge_attr[e] @ Wa.T
                   + (agg[src_e] - excl[e]) @ Wm.T + b_upd)
  with W_upd = [Wx | Wa | Wm] split along columns (64 | 16 | 64).

Decomposition (no cross-core communication at all):
    node_term[v] = x[v] @ Wx.T + agg[v] @ Wm.T + b_upd
    out[e] = relu(node_term[src_e] + edge_attr[e] @ Wa.T - excl[e] @ Wm.T)

  Each core owns 5000 nodes. Edges are reverse pairs (e <-> e +/- E/2),
  so for out-edge e = rev(f), excl[e] = msg[f] (plus rare duplicate-pair
  corrections) and src_e = tgt_f. Sorting in-edges by tgt gives one
  stream that serves both passes:
    pass 1: msg(f) -> one-hot matmul segment-sum -> agg -> node_term
    pass 2: out[rev(f)] = relu(nt[tgt_f] + attrW[rev(f)] - msg(f)@Wm.T)
  node_term rows are delivered by a host-built one-hot matmul (U2), so
  there are no gathers. ~500 duplicate-pair corrections go through 128
  "special" node_term rows computed on device and a fix-up group whose
  outputs the host splices in.

Matmul dtypes: bf16 for the big per-edge streams (inputs host-cast),
float32r (1.6e-4) for node_term math. All accumulation is fp32 PSUM.
"""

import numpy as np
import ml_dtypes

import concourse.bacc as bacc
import concourse.bass as bass
import concourse.mybir as mybir
import concourse.tile as tile
from concourse.bass_utils import run_bass_kernel_spmd

F32 = mybir.dt.float32
F32R = mybir.dt.float32r
BF16 = mybir.dt.bfloat16
I32 = mybir.dt.int32
ALU = mybir.AluOpType
ACTF = mybir.ActivationFunctionType
NPBF = ml_dtypes.bfloat16

N = 40000
E = 800000
E2 = E // 2
H = 64
A = 16
NC = 8
P = 128

NPC = N // NC           # 5000 nodes per core
NBLK = 40               # 128-node blocks per core
NPC_PAD = NBLK * P      # 5120
SPEC_CAP = P            # special (correction) rows per core
NT_ROWS = NPC_PAD + SPEC_CAP

_CACHE = {}
_DEBUG_NT = False


def _build(k_blk: int):
    nch = NBLK * k_blk              # chunks per core (both passes)
    l1 = nch * P                    # padded edges per core
    assert nch % 2 == 0
    hch = nch // 2                  # chunks per partition-half of eh

    nc = bacc.Bacc("TRN2", target_bir_lowering=False, debug=False,
                   num_devices=NC)

    def inp(name, shape, dtype):
        return nc.dram_tensor(name, shape, dtype, kind="ExternalInput").ap()

    # eh (in-edges, tgt-sorted, feature-major): chunks 0..hch-1 on
    # partitions 0:64, chunks hch.. on partitions 64:128.
    eh2 = inp("eh2", [P, hch * P], BF16)
    tgt_rel = inp("tgt_rel", [P, nch], F32)
    attr_T = inp("attr_T", [A, l1], BF16)      # edge_attr of rev(f), T
    U2 = inp("U2", [P, l1], BF16)              # one-hot src_rel columns
    xT_own = inp("xT_own", [H, NPC_PAD], F32R)
    ehF_T = inp("ehF_T", [H, P], BF16)         # correction source rows
    ehRF_T = inp("ehRF_T", [H, P], BF16)       # eh[rev(affected e)], T
    attrF_T = inp("attrF_T", [A, P], BF16)
    Sneg = inp("Sneg", [P, P], F32R)
    didx = inp("didx", [P, 1], I32)
    Wmsg2 = inp("Wmsg2", [P, H], BF16)         # W_msg.T doubled (2x64)
    Wua = inp("Wua", [A, H], BF16)
    negWum = inp("negWum", [H, H], BF16)
    Wstack = inp("Wstack", [H + A, H], BF16)   # [negWum ; Wua]
    Wum = inp("Wum", [H, H], F32R)
    Wux = inp("Wux", [H, H], F32R)
    bupd = inp("bupd", [1, H], F32R)
    ones1 = inp("ones1", [1, P], F32R)
    ident = inp("ident", [P, P], BF16)
    iota4 = inp("iota4", [P, 4 * P], BF16)

    outT = nc.dram_tensor("outT", [H, l1 + P], F32, kind="ExternalOutput").ap()
    nt_own = nc.dram_tensor("nt_own", [NT_ROWS, P], BF16).ap()
    nt_dump = (nc.dram_tensor("nt_dump", [NT_ROWS, P], BF16,
                              kind="ExternalOutput").ap()
               if _DEBUG_NT else None)

    with tile.TileContext(nc) as tc:
        with (
            tc.tile_pool(name="const", bufs=1) as cst,
            tc.tile_pool(name="sb", bufs=3) as sb,
            tc.tile_pool(name="stage", bufs=3) as stg,
            tc.tile_pool(name="ps_msg", bufs=2, space="PSUM") as ps_msg,
            tc.tile_pool(name="ps_agg", bufs=2, space="PSUM") as ps_agg,
            tc.tile_pool(name="ps_m", bufs=2, space="PSUM") as ps_m,
            tc.tile_pool(name="ps_o", bufs=2, space="PSUM") as ps_o,
        ):
            def load_const(name, ap_in, shape, dtype):
                t = cst.tile(shape, dtype, tag=name)
                nc.sync.dma_start(t[:], ap_in[:])
                return t

            eh_sb = load_const("c_eh2", eh2, [P, hch * P], BF16)
            tgt_rel_sb = load_const("c_tgtrel", tgt_rel, [P, nch], F32)
            xT_sb = load_const("c_xt", xT_own, [H, NPC_PAD], F32R)
            Wmsg2_sb = load_const("c_wmsg2", Wmsg2, [P, H], BF16)
            Wua_sb = load_const("c_wua", Wua, [A, H], BF16)
            negWum_sb = load_const("c_nwum", negWum, [H, H], BF16)
            Wstack_sb = load_const("c_wstack", Wstack, [H + A, H], BF16)
            Wum_sb = load_const("c_wum", Wum, [H, H], F32R)
            Wux_sb = load_const("c_wux", Wux, [H, H], F32R)
            bupd_sb = load_const("c_bupd", bupd, [1, H], F32R)
            ones1_sb = load_const("c_ones1", ones1, [1, P], F32R)
            ident_sb = load_const("c_ident", ident, [P, P], BF16)
            iota4_sb = load_const("c_iota4", iota4, [P, 4 * P], BF16)
            Sneg_sb = load_const("c_sneg", Sneg, [P, P], F32R)
            didx_sb = load_const("c_didx", didx, [P, 1], I32)
            ehF_sb = load_const("c_ehf", ehF_T, [H, P], BF16)
            ehRF_sb = load_const("c_ehrf", ehRF_T, [H, P], BF16)
            attrF_sb = load_const("c_attrf", attrF_T, [A, P], BF16)

            def ehsl(ch, w=P):
                half, col = (0, ch) if ch < hch else (64, ch - hch)
                return eh_sb[half:half + H, col * P:col * P + w]

            def wmsl(ch):
                half = 0 if ch < hch else 64
                return Wmsg2_sb[half:half + H, :]

            # b_upd broadcast to 128 partitions via K=1 matmul
            ps_b = ps_agg.tile([P, H], F32, tag="agg")
            nc.tensor.matmul(ps_b[:], lhsT=ones1_sb[:],
                             rhs=bupd_sb[:],
                             start=True, stop=True)
            b_bcast = cst.tile([P, H], F32, tag="c_bb")
            nc.vector.tensor_copy(b_bcast[:], ps_b[:])

            # ---- pass 1: msg -> agg -> node_term, per 128-node block ----
            for b in range(NBLK):
                agg_ps = ps_agg.tile([H, P], F32, tag="agg")
                i = 0
                while i < k_blk:
                    gw = min(4, k_blk - i)
                    msg4_ps = ps_msg.tile([P, 4 * H], F32, tag="msg")
                    for j in range(gw):
                        ch = b * k_blk + i + j
                        nc.tensor.matmul(msg4_ps[:, j * H:(j + 1) * H],
                                         lhsT=ehsl(ch), rhs=wmsl(ch),
                                         start=True, stop=True)
                    msg4_sb = sb.tile([P, 4 * H], BF16, tag="msg_sb")
                    nc.vector.tensor_scalar(out=msg4_sb[:, :gw * H],
                                            in0=msg4_ps[:, :gw * H],
                                            scalar1=0.0, scalar2=None,
                                            op0=ALU.max)
                    ch0 = b * k_blk + i
                    t4_sb = sb.tile([P, 4 * P], BF16, tag="t_sb")
                    trs = tgt_rel_sb[:, ch0:ch0 + gw]
                    tr_bc = bass.AP(trs.tensor, trs.offset,
                                    trs.ap[:1] + [[1, gw], [0, P]])
                    nc.vector.tensor_tensor(out=t4_sb[:, :gw * P],
                                            in0=iota4_sb[:, :gw * P],
                                            in1=tr_bc, op=ALU.is_equal)
                    for j in range(gw):
                        nc.tensor.matmul(agg_ps[:],
                                         lhsT=msg4_sb[:, j * H:(j + 1) * H],
                                         rhs=t4_sb[:, j * P:(j + 1) * P],
                                         start=(i + j == 0),
                                         stop=(i + j == k_blk - 1))
                    i += gw
                aggT_sb = sb.tile([H, P], F32R, tag="aggT_sb")
                nc.vector.tensor_copy(aggT_sb[:], agg_ps[:])
                nt_ps = ps_msg.tile([P, H], F32, tag="msg")
                nc.tensor.matmul(nt_ps[:], lhsT=aggT_sb[:],
                                 rhs=Wum_sb[:],
                                 start=True, stop=False)
                nc.tensor.matmul(nt_ps[:],
                                 lhsT=xT_sb[:, b * P:(b + 1) * P],
                                 rhs=Wux_sb[:],
                                 start=False, stop=True)
                nt_sb = sb.tile([P, P], BF16, tag="nt_sb")
                nc.gpsimd.memset(nt_sb[:, H:], 0.0)
                nc.vector.tensor_tensor(out=nt_sb[:, 0:H], in0=nt_ps[:],
                                        in1=b_bcast[:], op=ALU.add)
                nc.sync.dma_start(nt_own[b * P:(b + 1) * P, :], nt_sb[:])

            # ---- special (correction) rows ----
            mF_ps = ps_m.tile([H, P], F32, tag="m")
            nc.tensor.matmul(mF_ps[:], lhsT=Wmsg2_sb[0:H, :], rhs=ehF_sb[:],
                             start=True, stop=True)
            mFT_sb = sb.tile([H, P], F32R, tag="mFT_sb")
            nc.vector.tensor_scalar(out=mFT_sb[:], in0=mF_ps[:], scalar1=0.0,
                                    scalar2=None, op0=ALU.max)
            mV_ps = ps_msg.tile([P, H], F32, tag="msg")
            nc.tensor.matmul(mV_ps[:], lhsT=mFT_sb[:],
                             rhs=Wum_sb[:],
                             start=True, stop=True)
            mV_sb = sb.tile([P, H], F32R, tag="mV_sb")
            nc.vector.tensor_copy(mV_sb[:], mV_ps[:])
            ntgD_sb = sb.tile([P, P], BF16, tag="ntgD_sb")
            nc.gpsimd.indirect_dma_start(
                out=ntgD_sb[:], out_offset=None, in_=nt_own[:],
                in_offset=bass.IndirectOffsetOnAxis(ap=didx_sb[:, 0:1], axis=0),
            )
            ntgD_f = sb.tile([P, H], F32, tag="ntgD_f")
            nc.vector.tensor_copy(ntgD_f[:], ntgD_sb[:, 0:H])
            spec_ps = ps_agg.tile([P, H], F32, tag="agg")
            nc.tensor.matmul(spec_ps[:], lhsT=Sneg_sb[:],
                             rhs=mV_sb[:],
                             start=True, stop=True)
            spec_sb = sb.tile([P, P], BF16, tag="spec_sb")
            nc.gpsimd.memset(spec_sb[:, H:], 0.0)
            nc.vector.tensor_tensor(out=spec_sb[:, 0:H], in0=spec_ps[:],
                                    in1=ntgD_f[:], op=ALU.add)
            nc.sync.dma_start(nt_own[NPC_PAD:NPC_PAD + SPEC_CAP, :],
                              spec_sb[:])

            if nt_dump is not None:
                nc.sync.dma_start(nt_dump[:], nt_own[:])

            # ---- pass 2: out[rev(f)] per block, groups of <=4 chunks ----
            # stacked rhs: partitions 0:64 = relu(msg_rev)T, 64:80 = attrT
            for b in range(NBLK):
                ntb_sb = sb.tile([P, P], BF16, tag="ntb")
                nc.sync.dma_start(ntb_sb[:], nt_own[b * P:(b + 1) * P, :])
                i = 0
                while i < k_blk:
                    gw = min(4, k_blk - i)          # chunks in this group
                    w = gw * P
                    ch0 = b * k_blk + i
                    c0 = ch0 * P
                    m_ps = ps_m.tile([H, 4 * P], F32, tag="m")
                    nc.tensor.matmul(m_ps[:, 0:w], lhsT=wmsl(ch0),
                                     rhs=ehsl(ch0, w), start=True, stop=True)
                    sx_sb = stg.tile([H + A, 4 * P], BF16, tag="sx")
                    nc.scalar.activation(sx_sb[0:H, 0:w], m_ps[:, 0:w],
                                         ACTF.Relu)
                    nc.scalar.dma_start(sx_sb[H:H + A, 0:w],
                                        attr_T[:, c0:c0 + w])
                    u2_sb = stg.tile([P, 4 * P], BF16, tag="u2")
                    nc.sync.dma_start(u2_sb[:, 0:w], U2[:, c0:c0 + w])
                    o_ps = ps_o.tile([H, 4 * P], F32, tag="o")
                    nc.tensor.matmul(o_ps[:, 0:w], lhsT=Wstack_sb[:],
                                     rhs=sx_sb[:, 0:w],
                                     start=True, stop=False)
                    nc.tensor.matmul(o_ps[:, 0:w], lhsT=ntb_sb[:, 0:H],
                                     rhs=u2_sb[:, 0:w],
                                     start=False, stop=True)
                    outT_sb = sb.tile([H, 4 * P], F32, tag="outT")
                    nc.vector.tensor_scalar(out=outT_sb[:, 0:w],
                                            in0=o_ps[:, 0:w], scalar1=0.0,
                                            scalar2=None, op0=ALU.max)
                    nc.scalar.dma_start(outT[:, c0:c0 + w], outT_sb[:, 0:w])
                    i += gw

            # ---- fix-up group for the corrected edges ----
            ntf_sb = sb.tile([P, P], BF16, tag="ntb")
            nc.sync.dma_start(ntf_sb[:], nt_own[NPC_PAD:NPC_PAD + P, :])
            mf_ps = ps_m.tile([H, 4 * P], F32, tag="m")
            nc.tensor.matmul(mf_ps[:, 0:P], lhsT=Wmsg2_sb[0:H, :],
                             rhs=ehRF_sb[:], start=True, stop=True)
            mfT_sb = sb.tile([H, 4 * P], BF16, tag="mrevT")
            nc.scalar.activation(mfT_sb[:, 0:P], mf_ps[:, 0:P], ACTF.Relu)
            of_ps = ps_o.tile([H, 4 * P], F32, tag="o")
            nc.tensor.matmul(of_ps[:, 0:P], lhsT=Wua_sb[:], rhs=attrF_sb[:],
                             start=True, stop=False)
            nc.tensor.matmul(of_ps[:, 0:P], lhsT=negWum_sb[:],
                             rhs=mfT_sb[:, 0:P], start=False, stop=False)
            nc.tensor.matmul(of_ps[:, 0:P], lhsT=ntf_sb[:, 0:H],
                             rhs=ident_sb[:], start=False, stop=True)
            outF_sb = sb.tile([H, 4 * P], F32, tag="outT")
            nc.vector.tensor_scalar(out=outF_sb[:, 0:P], in0=of_ps[:, 0:P],
                                    scalar1=0.0, scalar2=None, op0=ALU.max)
            nc.sync.dma_start(outT[:, l1:l1 + P], outF_sb[:, 0:P])

    nc.compile()
    return nc


def _host_prep(x, edge_attr, edge_hidden, W_msg, b_msg, W_upd, b_upd,
               edge_index):
    src = np.asarray(edge_index[0], dtype=np.int64)
    tgt = np.asarray(edge_index[1], dtype=np.int64)
    eh = np.asarray(edge_hidden, dtype=np.float32)
    ea = np.asarray(edge_attr, dtype=np.float32)
    x = np.asarray(x, dtype=np.float32)
    W_msg = np.asarray(W_msg, dtype=np.float32)
    b_msg = np.asarray(b_msg, dtype=np.float32)
    W_upd = np.asarray(W_upd, dtype=np.float32)
    b_upd = np.asarray(b_upd, dtype=np.float32)
    assert not np.any(b_msg), "nonzero b_msg unsupported by this build"

    # ---- tgt-sort & per-(core, block) runs ----
    order = np.argsort(tgt, kind="stable")
    tgt_s = tgt[order]
    bnd = np.empty((NC, NBLK, 2), np.int64)
    for c in range(NC):
        for b in range(NBLK):
            lo_n = c * NPC + b * P
            hi_n = min(c * NPC + (b + 1) * P, (c + 1) * NPC)
            bnd[c, b] = (np.searchsorted(tgt_s, lo_n, "left"),
                         np.searchsorted(tgt_s, hi_n, "left"))
    runs = bnd[:, :, 1] - bnd[:, :, 0]
    k_blk = int(np.ceil(runs.max() / P))
    if k_blk % 2:
        k_blk += 1                      # nch even for the 2-half packing
    nch = NBLK * k_blk
    l1 = nch * P
    hch = nch // 2

    # ---- exclusion groups (reference's int logic) ----
    keys = tgt * N + src
    q = src * N + tgt
    order2 = np.argsort(keys, kind="stable")
    sk = keys[order2]
    lo2 = np.searchsorted(sk, q, "left")
    hi2 = np.searchsorted(sk, q, "right")
    eids = np.arange(E, dtype=np.int64)
    rev = np.where(eids < E2, eids + E2, eids - E2)
    simple = (hi2 - lo2 == 1) & (order2[lo2] == rev)
    affected = np.where(~simple)[0]

    Wmsg_io = np.ascontiguousarray(W_msg.T)         # [in, out]
    Wmsg2 = np.concatenate([Wmsg_io, Wmsg_io], axis=0).astype(NPBF)
    iota_t = np.tile(np.arange(P, dtype=np.float32), (P, 1))

    in_maps = []
    meta = []
    for c in range(NC):
        gl = np.zeros(l1, np.int64)      # in-edge f per padded position
        trel = np.full(l1, -1.0, np.float32)
        valid = np.zeros(l1, bool)
        for b in range(NBLK):
            lo, hi = bnd[c, b]
            n = hi - lo
            base = b * k_blk * P
            gl[base:base + n] = order[lo:hi]
            trel[base:base + n] = tgt_s[lo:hi] - (c * NPC + b * P)
            valid[base:base + n] = True

        ehp = eh[gl].astype(NPBF)                     # [l1, 64]
        eh2 = np.empty((P, hch * P), NPBF)
        eh2[0:H] = ehp[:hch * P].T
        eh2[H:P] = ehp[hch * P:].T

        tgt_rel = np.ascontiguousarray(
            trel.reshape(nch, P).T)

        # pass 2: out-edge e = rev(f); src_e = tgt_f
        el = rev[gl]
        attr_Tc = np.ascontiguousarray(ea[el].T).astype(NPBF)
        u2 = np.zeros((P, l1), np.float32)
        pos = np.arange(l1)
        tr = trel.astype(np.int64)
        u2[tr[valid], pos[valid]] = 1.0
        u2 = u2.astype(NPBF)

        xpad = np.zeros((NPC_PAD, H), np.float32)
        n_x = min(NPC_PAD, N - c * NPC)
        xpad[:n_x] = x[c * NPC:c * NPC + n_x]

        # corrections
        aff_c = affected[(src[affected] >= c * NPC)
                         & (src[affected] < (c + 1) * NPC)]
        f_list, s_cols = [], []
        for d, e in enumerate(aff_c):
            for f in order2[lo2[e]:hi2[e]]:
                if f != rev[e]:
                    f_list.append(f)
                    s_cols.append(d)
        assert len(aff_c) <= SPEC_CAP, len(aff_c)
        assert len(f_list) <= P, len(f_list)
        ehF = np.zeros((P, H), np.float32)
        if f_list:
            ehF[:len(f_list)] = eh[np.asarray(f_list)]
        ehRF = np.zeros((P, H), np.float32)
        attrF = np.zeros((P, A), np.float32)
        if len(aff_c):
            ehRF[:len(aff_c)] = eh[rev[aff_c]]
            attrF[:len(aff_c)] = ea[aff_c]
        Sneg = np.zeros((P, P), np.float32)
        for fi, d in enumerate(s_cols):
            Sneg[fi, d] = -1.0
        didx = np.zeros((P, 1), np.int32)
        didx[:len(aff_c), 0] = src[aff_c] - c * NPC

        in_maps.append({
            "eh2": eh2,
            "tgt_rel": tgt_rel,
            "attr_T": attr_Tc,
            "U2": u2,
            "xT_own": np.ascontiguousarray(xpad.T),
            "ehF_T": np.ascontiguousarray(ehF.T).astype(NPBF),
            "ehRF_T": np.ascontiguousarray(ehRF.T).astype(NPBF),
            "attrF_T": np.ascontiguousarray(attrF.T).astype(NPBF),
            "Sneg": Sneg,
            "didx": didx,
            "Wmsg2": Wmsg2,
            "Wua": np.ascontiguousarray(W_upd[:, H:H + A].T).astype(NPBF),
            "negWum": np.ascontiguousarray(-W_upd[:, H + A:].T).astype(NPBF),
            "Wstack": np.concatenate(
                [-W_upd[:, H + A:].T, W_upd[:, H:H + A].T],
                axis=0).astype(NPBF),
            "Wum": np.ascontiguousarray(W_upd[:, H + A:].T),
            "Wux": np.ascontiguousarray(W_upd[:, :H].T),
            "bupd": np.ascontiguousarray(b_upd[None, :]),
            "ones1": np.ones((1, P), np.float32),
            "ident": np.eye(P, dtype=np.float32).astype(NPBF),
            "iota4": np.tile(iota_t, (1, 4)).astype(NPBF),
        })
        meta.append({"el": el, "valid": valid, "aff_c": aff_c})
    return in_maps, meta, k_blk


def kernel(**inputs) -> np.ndarray:
    in_maps, meta, k_blk = _host_prep(**inputs)
    if k_blk not in _CACHE:
        _CACHE[k_blk] = _build(k_blk)
    nc = _CACHE[k_blk]
    res = run_bass_kernel_spmd(nc, in_maps, core_ids=list(range(NC)))
    l1 = NBLK * k_blk * P
    out = np.empty((E, H), np.float32)
    for c in range(NC):
        oT = res.results[c]["outT"]
        m = meta[c]
        out[m["el"][m["valid"]]] = oT[:, :l1].T[m["valid"]]
    for c in range(NC):
        oT = res.results[c]["outT"]
        aff_c = meta[c]["aff_c"]
        if len(aff_c):
            out[aff_c] = oT[:, l1:l1 + len(aff_c)].T
    return out



# revision 2
# speedup vs baseline: 2.3577x; 2.3577x over previous
"""Trainium2 Bass kernel for a directed MPNN layer (8 NeuronCores, SPMD).

Reference computation (per edge e = (src, tgt)):
    msg  = relu(edge_hidden @ W_msg.T + b_msg)                     (E, H)
    agg  = segment_sum(msg, tgt, N)                                (N, H)
    excl[e] = sum msg[f] over f with (tgt_f, src_f) == (src_e, tgt_e)
    out[e]  = relu(x[src_e] @ Wx.T + edge_attr[e] @ Wa.T
                   + (agg[src_e] - excl[e]) @ Wm.T + b_upd)
  with W_upd = [Wx | Wa | Wm] split along columns (64 | 16 | 64).

Decomposition (no cross-core communication):
    nt[v]  = xb[v] + agg[v] @ Wm.T          (xb = x @ Wx.T + b_upd, host)
    out[e] = relu(nt[src_e] + edge_attr[e] @ Wa.T - excl[e] @ Wm.T)
  Edges are reverse pairs; for out-edge e = rev(f), excl[e] = msg[f]
  (rare duplicate-pair corrections fixed in a small tail group).

Layout: each core owns 5000 nodes = 40 blocks of 128.  In-edges are
tgt-sorted and packed per block into k_blk chunks of 128 edge slots.
Chunks are PAIRED: pair p = chunks (2p, 2p+1) share 128 columns, with
the even chunk's features on partitions 0:64 and the odd chunk's on
64:128.  Per block (fused pass1+pass2, all intermediates in SBUF):
    msg   = eh_pair.T @ blockdiag(Wm)      per pair      (for agg)
    agg  += msg_chunk.T @ t4_chunk         per chunk     (one-hot scatter)
    nt    = [aggT;xbT].T @ [Wum;I]         per block
    msgT  = blockdiag(Wm).T @ eh_pair      per 4-pair group
    o2    = blockdiag(-Wum).T @ relu(msgT) + blockdiag(Wua).T @ attr2
            + nt.T one-hot-gathered via u2 (even rows 0:64, odd 64:128)
    out   = relu(o2)  -> bf16 -> HBM
One-hot matrices t4 (edge-part x node-free) and u2 (node-part x
edge-free) are built on host as int8 and cast to bf16 by SWDGE DMA.
"""

import numpy as np
import ml_dtypes

import concourse.bacc as bacc
import concourse.bass as bass
import concourse.mybir as mybir
import concourse.tile as tile
from concourse.bass_utils import run_bass_kernel_spmd

F32 = mybir.dt.float32
F32R = mybir.dt.float32r
BF16 = mybir.dt.bfloat16
I8 = mybir.dt.int8
I32 = mybir.dt.int32
ALU = mybir.AluOpType
ACTF = mybir.ActivationFunctionType
NPBF = ml_dtypes.bfloat16

N = 40000
E = 800000
E2 = E // 2
H = 64
A = 16
NC = 8
P = 128

NPC = N // NC           # 5000 nodes per core
NBLK = 40               # 128-node blocks per core
NPC_PAD = NBLK * P      # 5120
SPEC_CAP = P            # correction rows per core

_CACHE = {}


def _build(k_blk: int):
    assert k_blk % 2 == 0
    kp = k_blk // 2                 # pairs per block
    nch = NBLK * k_blk
    l1 = nch * P                    # padded edge slots per core
    bcol = kp * P                   # paired columns per block
    ncol = NBLK * bcol              # paired columns per core
    # pair groups per block (<=4 pairs so psum free dim <=512)
    grp = []
    p0 = 0
    while p0 < kp:
        gw = min(4, kp - p0)
        grp.append((p0, gw))
        p0 += gw

    nc = bacc.Bacc("TRN2", target_bir_lowering=False, debug=False,
                   num_devices=NC)

    def inp(name, shape, dtype):
        return nc.dram_tensor(name, shape, dtype, kind="ExternalInput").ap()

    eh2 = inp("eh2", [P, ncol], BF16)        # paired eh.T
    attr2 = inp("attr2", [2 * A, ncol], BF16)  # paired attr[rev].T
    t4i8 = inp("t4i8", [P, l1], I8)          # one-hot [edge-part, node]
    u2i8 = inp("u2i8", [P, l1], I8)          # one-hot [node-part, edge]
    xbT = inp("xbT", [H, NPC_PAD], F32R)     # (x@Wx.T + b_upd).T
    Wblk = inp("Wblk", [P, P], BF16)         # blockdiag(W_msg.T)
    nWumblk = inp("nWumblk", [P, P], BF16)   # blockdiag(-W_upd[:,H+A:].T)
    Wuablk = inp("Wuablk", [2 * A, P], BF16)  # blockdiag(W_upd[:,H:H+A].T)
    Wum = inp("Wum", [H, H], F32R)           # W_upd[:, H+A:].T
    I64 = inp("I64", [H, H], F32R)
    # correction constants
    Wio64 = inp("Wio64", [H, H], BF16)       # W_msg.T
    nWum64 = inp("nWum64", [H, H], BF16)
    Wua64 = inp("Wua64", [A, H], BF16)
    ehF_T = inp("ehF_T", [H, P], BF16)
    ehRF_T = inp("ehRF_T", [H, P], BF16)
    attrF_T = inp("attrF_T", [A, P], BF16)
    Sneg = inp("Sneg", [P, P], F32R)
    didx = inp("didx", [P, 1], I32)
    ident = inp("ident", [P, P], BF16)

    outT2 = nc.dram_tensor("outT2", [P, ncol], BF16,
                           kind="ExternalOutput").ap()
    outF = nc.dram_tensor("outF", [H, P], F32, kind="ExternalOutput").ap()
    nt_own = nc.dram_tensor("nt_own", [NPC_PAD, H], BF16).ap()

    with tile.TileContext(nc) as tc:
        with (
            tc.tile_pool(name="const", bufs=1) as cst,
            tc.tile_pool(name="peh", bufs=3) as peh,
            tc.tile_pool(name="pu2", bufs=3) as pu2,
            tc.tile_pool(name="pt4", bufs=3) as pt4,
            tc.tile_pool(name="pat", bufs=3) as pat,
            tc.tile_pool(name="pmsg", bufs=2) as pmsg,
            tc.tile_pool(name="psxm", bufs=3) as psxm,
            tc.tile_pool(name="pout", bufs=3) as pout,
            tc.tile_pool(name="pagg", bufs=2) as pagg,
            tc.tile_pool(name="pnt", bufs=1) as pnt,
            tc.tile_pool(name="ps_msg", bufs=2, space="PSUM") as ps_msg,
            tc.tile_pool(name="ps_agg", bufs=1, space="PSUM") as ps_agg,
            tc.tile_pool(name="ps_mT", bufs=2, space="PSUM") as ps_mT,
            tc.tile_pool(name="ps_o", bufs=2, space="PSUM") as ps_o,
            tc.tile_pool(name="ps_nt", bufs=1, space="PSUM") as ps_nt,
        ):
            def load_const(name, ap_in, shape, dtype):
                t = cst.tile(shape, dtype, tag=name)
                nc.sync.dma_start(t[:], ap_in[:])
                return t

            Wblk_sb = load_const("c_wblk", Wblk, [P, P], BF16)
            nWum_sb = load_const("c_nwum", nWumblk, [P, P], BF16)
            Wua_sb = load_const("c_wua", Wuablk, [2 * A, P], BF16)
            Wum_sb = load_const("c_wum", Wum, [H, H], F32R)
            I64_sb = load_const("c_i64", I64, [H, H], F32R)
            xbT_sb = load_const("c_xbt", xbT, [H, NPC_PAD], F32R)
            Wio64_sb = load_const("c_wio64", Wio64, [H, H], BF16)
            nWum64_sb = load_const("c_nwum64", nWum64, [H, H], BF16)
            Wua64_sb = load_const("c_wua64", Wua64, [A, H], BF16)
            ehF_sb = load_const("c_ehf", ehF_T, [H, P], BF16)
            ehRF_sb = load_const("c_ehrf", ehRF_T, [H, P], BF16)
            attrF_sb = load_const("c_attrf", attrF_T, [A, P], BF16)
            Sneg_sb = load_const("c_sneg", Sneg, [P, P], F32R)
            didx_sb = load_const("c_didx", didx, [P, 1], I32)
            ident_sb = load_const("c_ident", ident, [P, P], BF16)

            nt_all = pnt.tile([P, NBLK * H], BF16, tag="nt_all")

            state = {}

            def pass1(b):
                c0 = b * bcol
                eh_t = peh.tile([P, bcol], BF16, tag="eh")
                nc.sync.dma_start(eh_t[:], eh2[:, c0:c0 + bcol])
                t4_t = pt4.tile([P, 2 * bcol], BF16, tag="t4")
                nc.gpsimd.dma_start(out=t4_t[:],
                                    in_=t4i8[:, 2 * c0:2 * c0 + 2 * bcol])
                u2_t = pu2.tile([P, 2 * bcol], BF16, tag="u2")
                nc.gpsimd.dma_start(out=u2_t[:],
                                    in_=u2i8[:, 2 * c0:2 * c0 + 2 * bcol])
                at_t = pat.tile([2 * A, bcol], BF16, tag="attr")
                nc.sync.dma_start(at_t[:], attr2[:, c0:c0 + bcol])

                msg_sb = pmsg.tile([P, bcol], BF16, tag="msg")
                for (g0, gw) in grp:
                    mp = ps_msg.tile([P, 512], F32, tag="msgps")
                    for j in range(gw):
                        nc.tensor.matmul(
                            mp[:, j * P:(j + 1) * P],
                            lhsT=eh_t[:, (g0 + j) * P:(g0 + j + 1) * P],
                            rhs=Wblk_sb[:], start=True, stop=True)
                    nc.vector.tensor_scalar(
                        out=msg_sb[:, g0 * P:(g0 + gw) * P],
                        in0=mp[:, :gw * P], scalar1=0.0, scalar2=None,
                        op0=ALU.max)
                agg_ps = ps_agg.tile([H, P], F32, tag="aggps")
                for c in range(k_blk):
                    pr, par = c // 2, c % 2
                    m0 = pr * P + par * H
                    nc.tensor.matmul(agg_ps[:],
                                     lhsT=msg_sb[:, m0:m0 + H],
                                     rhs=t4_t[:, c * P:(c + 1) * P],
                                     start=(c == 0), stop=(c == k_blk - 1))
                aggT_sb = pagg.tile([H, P], F32R, tag="aggT")
                nc.vector.tensor_copy(aggT_sb[:], agg_ps[:])
                nt_ps = ps_nt.tile([P, H], F32, tag="ntps")
                nc.tensor.matmul(nt_ps[:], lhsT=aggT_sb[:], rhs=Wum_sb[:],
                                 start=True, stop=False)
                nc.tensor.matmul(nt_ps[:],
                                 lhsT=xbT_sb[:, b * P:(b + 1) * P],
                                 rhs=I64_sb[:], start=False, stop=True)
                nc.vector.tensor_copy(nt_all[:, b * H:(b + 1) * H], nt_ps[:])
                state[b] = (eh_t, u2_t, at_t)

            def pass2(b):
                c0 = b * bcol
                eh_t, u2_t, at_t = state.pop(b)
                for (g0, gw) in grp:
                    w = gw * P
                    s0 = g0 * P
                    mt = ps_mT.tile([P, 512], F32, tag="mtps")
                    nc.tensor.matmul(mt[:, :w], lhsT=Wblk_sb[:],
                                     rhs=eh_t[:, s0:s0 + w],
                                     start=True, stop=True)
                    sxm = psxm.tile([P, 512], BF16, tag="sxm")
                    nc.scalar.activation(sxm[:, :w], mt[:, :w], ACTF.Relu)
                    o2 = ps_o.tile([P, 512], F32, tag="o2ps")
                    nc.tensor.matmul(o2[:, :w], lhsT=nWum_sb[:],
                                     rhs=sxm[:, :w], start=True, stop=False)
                    nc.tensor.matmul(o2[:, :w], lhsT=Wua_sb[:],
                                     rhs=at_t[:, s0:s0 + w],
                                     start=False, stop=False)
                    ntb = nt_all[:, b * H:(b + 1) * H]
                    nc.tensor.matmul(o2[0:H, :w], lhsT=ntb,
                                     rhs=u2_t[:, s0:s0 + w],
                                     start=False, stop=False)
                    nc.tensor.matmul(o2[H:P, :w], lhsT=ntb,
                                     rhs=u2_t[:, bcol + s0:bcol + s0 + w],
                                     start=False, stop=True)
                    oT = pout.tile([P, 512], BF16, tag="oT")
                    nc.scalar.activation(oT[:, :w], o2[:, :w], ACTF.Relu)
                    nc.sync.dma_start(outT2[:, c0 + s0:c0 + s0 + w],
                                      oT[:, :w])

            for b in range(NBLK):
                pass1(b)
                if b >= 1:
                    pass2(b - 1)
            pass2(NBLK - 1)

            # ---- correction tail ----
            # nt_all -> nt_own DRAM ([b*128+p, h] <- nt_all[p, b*64+h])
            nt_dst = bass.AP(nt_own.tensor, nt_own.offset,
                             [[H, P], [P * H, NBLK], [1, H]])
            nc.sync.dma_start(nt_dst, nt_all[:])
            # gather nt rows for affected sources
            ntgD_sb = pagg.tile([P, H], BF16, tag="ntgD")
            nc.gpsimd.indirect_dma_start(
                out=ntgD_sb[:], out_offset=None, in_=nt_own[:],
                in_offset=bass.IndirectOffsetOnAxis(ap=didx_sb[:, 0:1],
                                                    axis=0),
            )
            ntgD_f = pagg.tile([P, H], F32, tag="ntgDf")
            nc.vector.tensor_copy(ntgD_f[:], ntgD_sb[:])
            # mV = relu(msg_F) @ Wum for the excess group members
            mF_ps = ps_mT.tile([H, P], F32, tag="mtps")
            nc.tensor.matmul(mF_ps[:], lhsT=Wio64_sb[:], rhs=ehF_sb[:],
                             start=True, stop=True)
            mFT_sb = pagg.tile([H, P], F32R, tag="mFT")
            nc.vector.tensor_scalar(out=mFT_sb[:], in0=mF_ps[:], scalar1=0.0,
                                    scalar2=None, op0=ALU.max)
            mV_ps = ps_nt.tile([P, H], F32, tag="ntps")
            nc.tensor.matmul(mV_ps[:], lhsT=mFT_sb[:], rhs=Wum_sb[:],
                             start=True, stop=True)
            mV_sb = pagg.tile([P, H], F32R, tag="mV")
            nc.vector.tensor_copy(mV_sb[:], mV_ps[:])
            # spec rows: nt[didx] - sum_excess msg@Wum
            spec_ps = ps_nt.tile([P, H], F32, tag="ntps")
            nc.tensor.matmul(spec_ps[:], lhsT=Sneg_sb[:], rhs=mV_sb[:],
                             start=True, stop=True)
            spec_sb = pagg.tile([P, H], BF16, tag="spec")
            nc.vector.tensor_tensor(out=spec_sb[:], in0=spec_ps[:],
                                    in1=ntgD_f[:], op=ALU.add)
            # fix-up outputs
            mf_ps = ps_mT.tile([H, P], F32, tag="mtps")
            nc.tensor.matmul(mf_ps[:], lhsT=Wio64_sb[:], rhs=ehRF_sb[:],
                             start=True, stop=True)
            mfT_sb = pagg.tile([H, P], BF16, tag="mfT")
            nc.scalar.activation(mfT_sb[:], mf_ps[:], ACTF.Relu)
            of_ps = ps_mT.tile([H, P], F32, tag="mtps")
            nc.tensor.matmul(of_ps[:], lhsT=Wua64_sb[:], rhs=attrF_sb[:],
                             start=True, stop=False)
            nc.tensor.matmul(of_ps[:], lhsT=nWum64_sb[:], rhs=mfT_sb[:],
                             start=False, stop=False)
            nc.tensor.matmul(of_ps[:], lhsT=spec_sb[:], rhs=ident_sb[:],
                             start=False, stop=True)
            outF_sb = pagg.tile([H, P], F32, tag="outF")
            nc.vector.tensor_scalar(out=outF_sb[:], in0=of_ps[:], scalar1=0.0,
                                    scalar2=None, op0=ALU.max)
            nc.sync.dma_start(outF[:], outF_sb[:])

    nc.compile()
    return nc


def _host_prep(x, edge_attr, edge_hidden, W_msg, b_msg, W_upd, b_upd,
               edge_index):
    src = np.asarray(edge_index[0], dtype=np.int64)
    tgt = np.asarray(edge_index[1], dtype=np.int64)
    eh = np.asarray(edge_hidden, dtype=np.float32)
    ea = np.asarray(edge_attr, dtype=np.float32)
    x = np.asarray(x, dtype=np.float32)
    W_msg = np.asarray(W_msg, dtype=np.float32)
    b_msg = np.asarray(b_msg, dtype=np.float32)
    W_upd = np.asarray(W_upd, dtype=np.float32)
    b_upd = np.asarray(b_upd, dtype=np.float32)
    assert not np.any(b_msg), "nonzero b_msg unsupported by this build"

    # ---- tgt-sort & per-(core, block) runs ----
    order = np.argsort(tgt, kind="stable")
    tgt_s = tgt[order]
    bnd = np.empty((NC, NBLK, 2), np.int64)
    for c in range(NC):
        for b in range(NBLK):
            lo_n = c * NPC + b * P
            hi_n = min(c * NPC + (b + 1) * P, (c + 1) * NPC)
            bnd[c, b] = (np.searchsorted(tgt_s, lo_n, "left"),
                         np.searchsorted(tgt_s, hi_n, "left"))
    runs = bnd[:, :, 1] - bnd[:, :, 0]
    k_blk = int(np.ceil(runs.max() / P))
    if k_blk % 2:
        k_blk += 1
    kp = k_blk // 2
    nch = NBLK * k_blk
    l1 = nch * P
    bcol = kp * P
    ncol = NBLK * bcol

    # ---- exclusion groups (reference's int logic) ----
    keys = tgt * N + src
    q = src * N + tgt
    order2 = np.argsort(keys, kind="stable")
    sk = keys[order2]
    lo2 = np.searchsorted(sk, q, "left")
    hi2 = np.searchsorted(sk, q, "right")
    eids = np.arange(E, dtype=np.int64)
    rev = np.where(eids < E2, eids + E2, eids - E2)
    simple = (hi2 - lo2 == 1) & (order2[lo2] == rev)
    affected = np.where(~simple)[0]

    Wmsg_io = np.ascontiguousarray(W_msg.T)              # [in, out]
    Wum_io = np.ascontiguousarray(W_upd[:, H + A:].T)    # [in, out]
    Wua_io = np.ascontiguousarray(W_upd[:, H:H + A].T)   # [16, 64]

    def blockdiag(w):
        k, m = w.shape
        out = np.zeros((2 * k, 2 * m), np.float32)
        out[:k, :m] = w
        out[k:, m:] = w
        return out

    Wblk = blockdiag(Wmsg_io).astype(NPBF)
    nWumblk = blockdiag(-Wum_io).astype(NPBF)
    Wuablk = blockdiag(Wua_io).astype(NPBF)

    xb = (x @ W_upd[:, :H].T + b_upd).astype(np.float32)  # [N, 64]

    in_maps = []
    meta = []
    slots = np.arange(l1)
    for c in range(NC):
        gl = np.zeros(l1, np.int64)
        trel = np.full(l1, -1, np.int64)
        valid = np.zeros(l1, bool)
        for b in range(NBLK):
            lo, hi = bnd[c, b]
            n = hi - lo
            base = b * k_blk * P
            gl[base:base + n] = order[lo:hi]
            trel[base:base + n] = tgt_s[lo:hi] - (c * NPC + b * P)
            valid[base:base + n] = True

        ehp = eh[gl].astype(NPBF)                         # [l1, 64]
        eh2 = np.ascontiguousarray(
            ehp.reshape(nch // 2, 2, P, H).transpose(1, 3, 0, 2)
            .reshape(P, ncol))

        el = rev[gl]
        attr2 = np.ascontiguousarray(
            ea[el].astype(NPBF).reshape(nch // 2, 2, P, A)
            .transpose(1, 3, 0, 2).reshape(2 * A, ncol))

        t4i8 = np.zeros((P, l1), np.int8)
        sv = slots[valid]
        t4i8[sv % P, (sv // P) * P + trel[sv]] = 1

        u2i8 = np.zeros((P, l1), np.int8)
        ch = sv // P
        blk = ch // k_blk
        par = ch % 2
        pr_in_b = (ch % k_blk) // 2
        ucol = blk * (2 * bcol) + par * bcol + pr_in_b * P + sv % P
        u2i8[trel[sv], ucol] = 1

        xpad = np.zeros((NPC_PAD, H), np.float32)
        n_x = min(NPC_PAD, N - c * NPC)
        xpad[:n_x] = xb[c * NPC:c * NPC + n_x]

        # corrections
        aff_c = affected[(src[affected] >= c * NPC)
                         & (src[affected] < (c + 1) * NPC)]
        f_list, s_cols = [], []
        for d, e in enumerate(aff_c):
            for f in order2[lo2[e]:hi2[e]]:
                if f != rev[e]:
                    f_list.append(f)
                    s_cols.append(d)
        assert len(aff_c) <= SPEC_CAP, len(aff_c)
        assert len(f_list) <= P, len(f_list)
        ehF = np.zeros((P, H), np.float32)
        if f_list:
            ehF[:len(f_list)] = eh[np.asarray(f_list)]
        ehRF = np.zeros((P, H), np.float32)
        attrF = np.zeros((P, A), np.float32)
        if len(aff_c):
            ehRF[:len(aff_c)] = eh[rev[aff_c]]
            attrF[:len(aff_c)] = ea[aff_c]
        Sneg = np.zeros((P, P), np.float32)
        for fi, d in enumerate(s_cols):
            Sneg[fi, d] = -1.0
        didx = np.zeros((P, 1), np.int32)
        didx[:len(aff_c), 0] = src[aff_c] - c * NPC

        in_maps.append({
            "eh2": eh2,
            "attr2": attr2,
            "t4i8": t4i8,
            "u2i8": u2i8,
            "xbT": np.ascontiguousarray(xpad.T),
            "Wblk": Wblk,
            "nWumblk": nWumblk,
            "Wuablk": Wuablk,
            "Wum": Wum_io,
            "I64": np.eye(H, dtype=np.float32),
            "Wio64": Wmsg_io.astype(NPBF),
            "nWum64": (-Wum_io).astype(NPBF),
            "Wua64": Wua_io.astype(NPBF),
            "ehF_T": np.ascontiguousarray(ehF.T).astype(NPBF),
            "ehRF_T": np.ascontiguousarray(ehRF.T).astype(NPBF),
            "attrF_T": np.ascontiguousarray(attrF.T).astype(NPBF),
            "Sneg": Sneg,
            "didx": didx,
            "ident": np.eye(P, dtype=np.float32).astype(NPBF),
        })
        meta.append({"el": el, "valid": valid, "aff_c": aff_c})
    return in_maps, meta, k_blk


def kernel(**inputs) -> np.ndarray:
    in_maps, meta, k_blk = _host_prep(**inputs)
    if k_blk not in _CACHE:
        _CACHE[k_blk] = _build(k_blk)
    nc = _CACHE[k_blk]
    res = run_bass_kernel_spmd(nc, in_maps, core_ids=list(range(NC)))
    nch = NBLK * k_blk
    l1 = nch * P
    out = np.empty((E, H), np.float32)
    for c in range(NC):
        oT = res.results[c]["outT2"]          # [128, ncol] bf16
        per_slot = (oT.astype(np.float32)
                    .reshape(2, H, nch // 2, P).transpose(2, 0, 3, 1)
                    .reshape(l1, H))
        m = meta[c]
        out[m["el"][m["valid"]]] = per_slot[m["valid"]]
    for c in range(NC):
        aff_c = meta[c]["aff_c"]
        if len(aff_c):
            oF = res.results[c]["outF"]       # [64, 128] f32
            out[aff_c] = oF[:, :len(aff_c)].T
    return out


# revision 8
# speedup vs baseline: 2.4314x; 1.0312x over previous
"""Trainium2 Bass kernel for a directed MPNN layer (8 NeuronCores, SPMD).

Reference computation (per edge e = (src, tgt)):
    msg  = relu(edge_hidden @ W_msg.T + b_msg)                     (E, H)
    agg  = segment_sum(msg, tgt, N)                                (N, H)
    excl[e] = sum msg[f] over f with (tgt_f, src_f) == (src_e, tgt_e)
    out[e]  = relu(x[src_e] @ Wx.T + edge_attr[e] @ Wa.T
                   + (agg[src_e] - excl[e]) @ Wm.T + b_upd)
  with W_upd = [Wx | Wa | Wm] split along columns (64 | 16 | 64).

Decomposition (no cross-core communication):
    nt[v]  = xb[v] + agg[v] @ Wm.T          (xb = x @ Wx.T + b_upd, host)
    out[e] = relu(nt[src_e] + edge_attr[e] @ Wa.T - excl[e] @ Wm.T)
  Edges are reverse pairs; for out-edge e = rev(f), excl[e] = msg[f]
  (rare duplicate-pair corrections fixed in a small tail group).

Layout: each core owns 5000 nodes = 40 blocks of 128.  In-edges are
tgt-sorted and packed per block into k_blk chunks of 128 edge slots.
Chunks are PAIRED: pair p = chunks (2p, 2p+1) share 128 columns, with
the even chunk's features on partitions 0:64 and the odd chunk's on
64:128.  Per block (fused pass1+pass2, all intermediates in SBUF):
    msg   = eh_pair.T @ blockdiag(Wm)      per pair      (for agg)
    agg  += msg_chunk.T @ t4_chunk         per chunk     (one-hot scatter)
    nt    = [aggT;xbT].T @ [Wum;I]         per block
    msgT  = blockdiag(Wm).T @ eh_pair      per 4-pair group
    o2    = blockdiag(-Wum).T @ relu(msgT) + blockdiag(Wua).T @ attr2
            + nt.T one-hot-gathered via u2 (even rows 0:64, odd 64:128)
    out   = relu(o2)  -> bf16 -> HBM
One-hot matrices t4 (edge-part x node-free) and u2 (node-part x
edge-free) are built on host as int8 and cast to bf16 by SWDGE DMA.
"""

import numpy as np
import ml_dtypes

import concourse.bacc as bacc
import concourse.bass as bass
import concourse.mybir as mybir
import concourse.tile as tile
from concourse.bass_utils import run_bass_kernel_spmd

F32 = mybir.dt.float32
F32R = mybir.dt.float32r
BF16 = mybir.dt.bfloat16
I8 = mybir.dt.int8
I32 = mybir.dt.int32
ALU = mybir.AluOpType
ACTF = mybir.ActivationFunctionType
NPBF = ml_dtypes.bfloat16

N = 40000
E = 800000
E2 = E // 2
H = 64
A = 16
NC = 8
P = 128

NPC = N // NC           # 5000 nodes per core
NBLK = 40               # 128-node blocks per core
NPC_PAD = NBLK * P      # 5120
SPEC_CAP = P            # correction rows per core

_CACHE = {}


def _build(k_blk: int):
    assert k_blk % 2 == 0
    kp = k_blk // 2                 # pairs per block
    nch = NBLK * k_blk
    l1 = nch * P                    # padded edge slots per core
    bcol = kp * P                   # paired columns per block
    ncol = NBLK * bcol              # paired columns per core
    # pair groups per block (<=4 pairs so psum free dim <=512)
    grp = []
    p0 = 0
    while p0 < kp:
        gw = min(4, kp - p0)
        grp.append((p0, gw))
        p0 += gw

    nc = bacc.Bacc("TRN2", target_bir_lowering=False, debug=False,
                   num_devices=NC)

    def inp(name, shape, dtype):
        return nc.dram_tensor(name, shape, dtype, kind="ExternalInput").ap()

    eh2 = inp("eh2", [P, ncol], BF16)        # paired eh.T
    attr2 = inp("attr2", [2 * A, ncol], BF16)  # paired attr[rev].T
    t4i8 = inp("t4i8", [P, l1], I8)          # one-hot [edge-part, node]
    u2i8 = inp("u2i8", [P, l1], I8)          # one-hot [node-part, edge]
    xbT = inp("xbT", [H, NPC_PAD], F32R)     # (x@Wx.T + b_upd).T
    Wblk = inp("Wblk", [P, P], BF16)         # blockdiag(W_msg.T)
    nWumblk = inp("nWumblk", [P, P], BF16)   # blockdiag(-W_upd[:,H+A:].T)
    Wuablk = inp("Wuablk", [2 * A, P], BF16)  # blockdiag(W_upd[:,H:H+A].T)
    Wum = inp("Wum", [H, H], F32R)           # W_upd[:, H+A:].T
    I64 = inp("I64", [H, H], F32R)
    # correction constants
    Wio64 = inp("Wio64", [H, H], BF16)       # W_msg.T
    nWum64 = inp("nWum64", [H, H], BF16)
    Wua64 = inp("Wua64", [A, H], BF16)
    ehF_T = inp("ehF_T", [H, P], BF16)
    ehRF_T = inp("ehRF_T", [H, P], BF16)
    attrF_T = inp("attrF_T", [A, P], BF16)
    Sneg = inp("Sneg", [P, P], F32R)
    didx = inp("didx", [P, 1], I32)
    ident = inp("ident", [P, P], BF16)

    outT2 = nc.dram_tensor("outT2", [P, ncol], BF16,
                           kind="ExternalOutput").ap()
    outF = nc.dram_tensor("outF", [H, P], F32, kind="ExternalOutput").ap()
    nt_own = nc.dram_tensor("nt_own", [NPC_PAD, H], BF16).ap()

    with tile.TileContext(nc) as tc:
        with (
            tc.tile_pool(name="const", bufs=1) as cst,
            tc.tile_pool(name="peh", bufs=3) as peh,
            tc.tile_pool(name="pu2", bufs=3) as pu2,
            tc.tile_pool(name="pt4", bufs=3) as pt4,
            tc.tile_pool(name="pat", bufs=3) as pat,
            tc.tile_pool(name="pmsg", bufs=2) as pmsg,
            tc.tile_pool(name="psxm", bufs=3) as psxm,
            tc.tile_pool(name="pout", bufs=3) as pout,
            tc.tile_pool(name="pagg", bufs=2) as pagg,
            tc.tile_pool(name="pnt", bufs=1) as pnt,
            tc.tile_pool(name="ps_msg", bufs=2, space="PSUM") as ps_msg,
            tc.tile_pool(name="ps_agg", bufs=1, space="PSUM") as ps_agg,
            tc.tile_pool(name="ps_mT", bufs=2, space="PSUM") as ps_mT,
            tc.tile_pool(name="ps_o", bufs=2, space="PSUM") as ps_o,
            tc.tile_pool(name="ps_nt", bufs=1, space="PSUM") as ps_nt,
        ):
            def load_const(name, ap_in, shape, dtype):
                t = cst.tile(shape, dtype, tag=name)
                nc.sync.dma_start(t[:], ap_in[:])
                return t

            Wblk_sb = load_const("c_wblk", Wblk, [P, P], BF16)
            nWum_sb = load_const("c_nwum", nWumblk, [P, P], BF16)
            Wua_sb = load_const("c_wua", Wuablk, [2 * A, P], BF16)
            Wum_sb = load_const("c_wum", Wum, [H, H], F32R)
            I64_sb = load_const("c_i64", I64, [H, H], F32R)
            xbT_sb = load_const("c_xbt", xbT, [H, NPC_PAD], F32R)
            Wio64_sb = load_const("c_wio64", Wio64, [H, H], BF16)
            nWum64_sb = load_const("c_nwum64", nWum64, [H, H], BF16)
            Wua64_sb = load_const("c_wua64", Wua64, [A, H], BF16)
            ehF_sb = load_const("c_ehf", ehF_T, [H, P], BF16)
            ehRF_sb = load_const("c_ehrf", ehRF_T, [H, P], BF16)
            attrF_sb = load_const("c_attrf", attrF_T, [A, P], BF16)
            Sneg_sb = load_const("c_sneg", Sneg, [P, P], F32R)
            didx_sb = load_const("c_didx", didx, [P, 1], I32)
            ident_sb = load_const("c_ident", ident, [P, P], BF16)

            nt_all = pnt.tile([P, NBLK * H], BF16, tag="nt_all")

            state = {}

            def pass1(b):
                c0 = b * bcol
                eh_t = peh.tile([P, bcol], BF16, tag="eh")
                nc.sync.dma_start(eh_t[:], eh2[:, c0:c0 + bcol])
                t4_t = pt4.tile([P, 2 * bcol], BF16, tag="t4")
                nc.gpsimd.dma_start(out=t4_t[:],
                                    in_=t4i8[:, 2 * c0:2 * c0 + 2 * bcol])
                u2_t = pu2.tile([P, 2 * bcol], BF16, tag="u2")
                nc.gpsimd.dma_start(out=u2_t[:],
                                    in_=u2i8[:, 2 * c0:2 * c0 + 2 * bcol])
                at_t = pat.tile([2 * A, bcol], BF16, tag="attr")
                nc.sync.dma_start(at_t[:], attr2[:, c0:c0 + bcol])

                msg_sb = pmsg.tile([P, bcol], BF16, tag="msg")
                for (g0, gw) in grp:
                    mp = ps_msg.tile([P, 512], F32, tag="msgps")
                    for j in range(gw):
                        nc.tensor.matmul(
                            mp[:, j * P:(j + 1) * P],
                            lhsT=eh_t[:, (g0 + j) * P:(g0 + j + 1) * P],
                            rhs=Wblk_sb[:], start=True, stop=True)
                    nc.vector.tensor_scalar(
                        out=msg_sb[:, g0 * P:(g0 + gw) * P],
                        in0=mp[:, :gw * P], scalar1=0.0, scalar2=None,
                        op0=ALU.max)
                agg_ps = ps_agg.tile([H, P], F32, tag="aggps")
                for c in range(k_blk):
                    pr, par = c // 2, c % 2
                    m0 = pr * P + par * H
                    nc.tensor.matmul(agg_ps[:],
                                     lhsT=msg_sb[:, m0:m0 + H],
                                     rhs=t4_t[:, c * P:(c + 1) * P],
                                     start=(c == 0), stop=(c == k_blk - 1))
                aggT_sb = pagg.tile([H, P], F32R, tag="aggT")
                nc.vector.tensor_copy(aggT_sb[:], agg_ps[:])
                nt_ps = ps_nt.tile([P, H], F32, tag="ntps")
                nc.tensor.matmul(nt_ps[:], lhsT=aggT_sb[:], rhs=Wum_sb[:],
                                 start=True, stop=False)
                nc.tensor.matmul(nt_ps[:],
                                 lhsT=xbT_sb[:, b * P:(b + 1) * P],
                                 rhs=I64_sb[:], start=False, stop=True)
                nc.vector.tensor_copy(nt_all[:, b * H:(b + 1) * H], nt_ps[:])
                state[b] = (eh_t, u2_t, at_t)

            def pass2(b):
                c0 = b * bcol
                eh_t, u2_t, at_t = state.pop(b)
                for (g0, gw) in grp:
                    w = gw * P
                    s0 = g0 * P
                    mt = ps_mT.tile([P, 512], F32, tag="mtps")
                    nc.tensor.matmul(mt[:, :w], lhsT=Wblk_sb[:],
                                     rhs=eh_t[:, s0:s0 + w],
                                     start=True, stop=True)
                    sxm = psxm.tile([P, 512], BF16, tag="sxm")
                    nc.scalar.activation(sxm[:, :w], mt[:, :w], ACTF.Relu)
                    o2 = ps_o.tile([P, 512], F32, tag="o2ps")
                    nc.tensor.matmul(o2[:, :w], lhsT=nWum_sb[:],
                                     rhs=sxm[:, :w], start=True, stop=False)
                    nc.tensor.matmul(o2[:, :w], lhsT=Wua_sb[:],
                                     rhs=at_t[:, s0:s0 + w],
                                     start=False, stop=False)
                    ntb = nt_all[:, b * H:(b + 1) * H]
                    nc.tensor.matmul(o2[0:H, :w], lhsT=ntb,
                                     rhs=u2_t[:, s0:s0 + w],
                                     start=False, stop=False)
                    nc.tensor.matmul(o2[H:P, :w], lhsT=ntb,
                                     rhs=u2_t[:, bcol + s0:bcol + s0 + w],
                                     start=False, stop=True)
                    oT = pout.tile([P, 512], BF16, tag="oT")
                    nc.vector.tensor_scalar(out=oT[:, :w], in0=o2[:, :w],
                                            scalar1=0.0, scalar2=None,
                                            op0=ALU.max)
                    nc.sync.dma_start(outT2[:, c0 + s0:c0 + s0 + w],
                                      oT[:, :w])

            for b in range(NBLK):
                pass1(b)
                if b >= 1:
                    pass2(b - 1)
            pass2(NBLK - 1)

            # ---- correction tail ----
            # nt_all -> nt_own DRAM ([b*128+p, h] <- nt_all[p, b*64+h])
            nt_dst = bass.AP(nt_own.tensor, nt_own.offset,
                             [[H, P], [P * H, NBLK], [1, H]])
            nc.sync.dma_start(nt_dst, nt_all[:])
            # gather nt rows for affected sources
            ntgD_sb = pagg.tile([P, H], BF16, tag="ntgD")
            nc.gpsimd.indirect_dma_start(
                out=ntgD_sb[:], out_offset=None, in_=nt_own[:],
                in_offset=bass.IndirectOffsetOnAxis(ap=didx_sb[:, 0:1],
                                                    axis=0),
            )
            ntgD_f = pagg.tile([P, H], F32, tag="ntgDf")
            nc.vector.tensor_copy(ntgD_f[:], ntgD_sb[:])
            # mV = relu(msg_F) @ Wum for the excess group members
            mF_ps = ps_mT.tile([H, P], F32, tag="mtps")
            nc.tensor.matmul(mF_ps[:], lhsT=Wio64_sb[:], rhs=ehF_sb[:],
                             start=True, stop=True)
            mFT_sb = pagg.tile([H, P], F32R, tag="mFT")
            nc.vector.tensor_scalar(out=mFT_sb[:], in0=mF_ps[:], scalar1=0.0,
                                    scalar2=None, op0=ALU.max)
            mV_ps = ps_nt.tile([P, H], F32, tag="ntps")
            nc.tensor.matmul(mV_ps[:], lhsT=mFT_sb[:], rhs=Wum_sb[:],
                             start=True, stop=True)
            mV_sb = pagg.tile([P, H], F32R, tag="mV")
            nc.vector.tensor_copy(mV_sb[:], mV_ps[:])
            # spec rows: nt[didx] - sum_excess msg@Wum
            spec_ps = ps_nt.tile([P, H], F32, tag="ntps")
            nc.tensor.matmul(spec_ps[:], lhsT=Sneg_sb[:], rhs=mV_sb[:],
                             start=True, stop=True)
            spec_sb = pagg.tile([P, H], BF16, tag="spec")
            nc.vector.tensor_tensor(out=spec_sb[:], in0=spec_ps[:],
                                    in1=ntgD_f[:], op=ALU.add)
            # fix-up outputs
            mf_ps = ps_mT.tile([H, P], F32, tag="mtps")
            nc.tensor.matmul(mf_ps[:], lhsT=Wio64_sb[:], rhs=ehRF_sb[:],
                             start=True, stop=True)
            mfT_sb = pagg.tile([H, P], BF16, tag="mfT")
            nc.scalar.activation(mfT_sb[:], mf_ps[:], ACTF.Relu)
            of_ps = ps_mT.tile([H, P], F32, tag="mtps")
            nc.tensor.matmul(of_ps[:], lhsT=Wua64_sb[:], rhs=attrF_sb[:],
                             start=True, stop=False)
            nc.tensor.matmul(of_ps[:], lhsT=nWum64_sb[:], rhs=mfT_sb[:],
                             start=False, stop=False)
            nc.tensor.matmul(of_ps[:], lhsT=spec_sb[:], rhs=ident_sb[:],
                             start=False, stop=True)
            outF_sb = pagg.tile([H, P], F32, tag="outF")
            nc.vector.tensor_scalar(out=outF_sb[:], in0=of_ps[:], scalar1=0.0,
                                    scalar2=None, op0=ALU.max)
            nc.sync.dma_start(outF[:], outF_sb[:])

    nc.compile()
    return nc


def _host_prep(x, edge_attr, edge_hidden, W_msg, b_msg, W_upd, b_upd,
               edge_index):
    src = np.asarray(edge_index[0], dtype=np.int64)
    tgt = np.asarray(edge_index[1], dtype=np.int64)
    eh = np.asarray(edge_hidden, dtype=np.float32)
    ea = np.asarray(edge_attr, dtype=np.float32)
    x = np.asarray(x, dtype=np.float32)
    W_msg = np.asarray(W_msg, dtype=np.float32)
    b_msg = np.asarray(b_msg, dtype=np.float32)
    W_upd = np.asarray(W_upd, dtype=np.float32)
    b_upd = np.asarray(b_upd, dtype=np.float32)
    assert not np.any(b_msg), "nonzero b_msg unsupported by this build"

    # ---- tgt-sort & per-(core, block) runs ----
    order = np.argsort(tgt, kind="stable")
    tgt_s = tgt[order]
    bnd = np.empty((NC, NBLK, 2), np.int64)
    for c in range(NC):
        for b in range(NBLK):
            lo_n = c * NPC + b * P
            hi_n = min(c * NPC + (b + 1) * P, (c + 1) * NPC)
            bnd[c, b] = (np.searchsorted(tgt_s, lo_n, "left"),
                         np.searchsorted(tgt_s, hi_n, "left"))
    runs = bnd[:, :, 1] - bnd[:, :, 0]
    k_blk = int(np.ceil(runs.max() / P))
    if k_blk % 2:
        k_blk += 1
    kp = k_blk // 2
    nch = NBLK * k_blk
    l1 = nch * P
    bcol = kp * P
    ncol = NBLK * bcol

    # ---- exclusion groups (reference's int logic) ----
    keys = tgt * N + src
    q = src * N + tgt
    order2 = np.argsort(keys, kind="stable")
    sk = keys[order2]
    lo2 = np.searchsorted(sk, q, "left")
    hi2 = np.searchsorted(sk, q, "right")
    eids = np.arange(E, dtype=np.int64)
    rev = np.where(eids < E2, eids + E2, eids - E2)
    simple = (hi2 - lo2 == 1) & (order2[lo2] == rev)
    affected = np.where(~simple)[0]

    Wmsg_io = np.ascontiguousarray(W_msg.T)              # [in, out]
    Wum_io = np.ascontiguousarray(W_upd[:, H + A:].T)    # [in, out]
    Wua_io = np.ascontiguousarray(W_upd[:, H:H + A].T)   # [16, 64]

    def blockdiag(w):
        k, m = w.shape
        out = np.zeros((2 * k, 2 * m), np.float32)
        out[:k, :m] = w
        out[k:, m:] = w
        return out

    Wblk = blockdiag(Wmsg_io).astype(NPBF)
    nWumblk = blockdiag(-Wum_io).astype(NPBF)
    Wuablk = blockdiag(Wua_io).astype(NPBF)

    xb = (x @ W_upd[:, :H].T + b_upd).astype(np.float32)  # [N, 64]

    in_maps = []
    meta = []
    slots = np.arange(l1)
    for c in range(NC):
        gl = np.zeros(l1, np.int64)
        trel = np.full(l1, -1, np.int64)
        valid = np.zeros(l1, bool)
        for b in range(NBLK):
            lo, hi = bnd[c, b]
            n = hi - lo
            base = b * k_blk * P
            gl[base:base + n] = order[lo:hi]
            trel[base:base + n] = tgt_s[lo:hi] - (c * NPC + b * P)
            valid[base:base + n] = True

        ehp = eh[gl].astype(NPBF)                         # [l1, 64]
        eh2 = np.ascontiguousarray(
            ehp.reshape(nch // 2, 2, P, H).transpose(1, 3, 0, 2)
            .reshape(P, ncol))

        el = rev[gl]
        attr2 = np.ascontiguousarray(
            ea[el].astype(NPBF).reshape(nch // 2, 2, P, A)
            .transpose(1, 3, 0, 2).reshape(2 * A, ncol))

        t4i8 = np.zeros((P, l1), np.int8)
        sv = slots[valid]
        t4i8[sv % P, (sv // P) * P + trel[sv]] = 1

        u2i8 = np.zeros((P, l1), np.int8)
        ch = sv // P
        blk = ch // k_blk
        par = ch % 2
        pr_in_b = (ch % k_blk) // 2
        ucol = blk * (2 * bcol) + par * bcol + pr_in_b * P + sv % P
        u2i8[trel[sv], ucol] = 1

        xpad = np.zeros((NPC_PAD, H), np.float32)
        n_x = min(NPC_PAD, N - c * NPC)
        xpad[:n_x] = xb[c * NPC:c * NPC + n_x]

        # corrections
        aff_c = affected[(src[affected] >= c * NPC)
                         & (src[affected] < (c + 1) * NPC)]
        f_list, s_cols = [], []
        for d, e in enumerate(aff_c):
            for f in order2[lo2[e]:hi2[e]]:
                if f != rev[e]:
                    f_list.append(f)
                    s_cols.append(d)
        assert len(aff_c) <= SPEC_CAP, len(aff_c)
        assert len(f_list) <= P, len(f_list)
        ehF = np.zeros((P, H), np.float32)
        if f_list:
            ehF[:len(f_list)] = eh[np.asarray(f_list)]
        ehRF = np.zeros((P, H), np.float32)
        attrF = np.zeros((P, A), np.float32)
        if len(aff_c):
            ehRF[:len(aff_c)] = eh[rev[aff_c]]
            attrF[:len(aff_c)] = ea[aff_c]
        Sneg = np.zeros((P, P), np.float32)
        for fi, d in enumerate(s_cols):
            Sneg[fi, d] = -1.0
        didx = np.zeros((P, 1), np.int32)
        didx[:len(aff_c), 0] = src[aff_c] - c * NPC

        in_maps.append({
            "eh2": eh2,
            "attr2": attr2,
            "t4i8": t4i8,
            "u2i8": u2i8,
            "xbT": np.ascontiguousarray(xpad.T),
            "Wblk": Wblk,
            "nWumblk": nWumblk,
            "Wuablk": Wuablk,
            "Wum": Wum_io,
            "I64": np.eye(H, dtype=np.float32),
            "Wio64": Wmsg_io.astype(NPBF),
            "nWum64": (-Wum_io).astype(NPBF),
            "Wua64": Wua_io.astype(NPBF),
            "ehF_T": np.ascontiguousarray(ehF.T).astype(NPBF),
            "ehRF_T": np.ascontiguousarray(ehRF.T).astype(NPBF),
            "attrF_T": np.ascontiguousarray(attrF.T).astype(NPBF),
            "Sneg": Sneg,
            "didx": didx,
            "ident": np.eye(P, dtype=np.float32).astype(NPBF),
        })
        meta.append({"el": el, "valid": valid, "aff_c": aff_c})
    return in_maps, meta, k_blk


def kernel(**inputs) -> np.ndarray:
    in_maps, meta, k_blk = _host_prep(**inputs)
    if k_blk not in _CACHE:
        _CACHE[k_blk] = _build(k_blk)
    nc = _CACHE[k_blk]
    res = run_bass_kernel_spmd(nc, in_maps, core_ids=list(range(NC)))
    nch = NBLK * k_blk
    l1 = nch * P
    out = np.empty((E, H), np.float32)
    for c in range(NC):
        oT = res.results[c]["outT2"]          # [128, ncol] bf16
        per_slot = (oT.astype(np.float32)
                    .reshape(2, H, nch // 2, P).transpose(2, 0, 3, 1)
                    .reshape(l1, H))
        m = meta[c]
        out[m["el"][m["valid"]]] = per_slot[m["valid"]]
    for c in range(NC):
        aff_c = meta[c]["aff_c"]
        if len(aff_c):
            oF = res.results[c]["outF"]       # [64, 128] f32
            out[aff_c] = oF[:, :len(aff_c)].T
    return out
